# revision 31
# baseline (speedup 1.0000x reference)
"""Trainium2 Bass kernel for nn_BioSimulator.

Math: out[b,h,w] = clip(2 * sum_n Bw[b,n] * exp(-((px-vx[n])^2+(py-vy[n])^2)
                        * deg2pix^2 / (2*sigma_px[b,n]^2)), 0, 1)

px varies only along w and py only along h, so the Gaussian separates:
    exp(-(dx^2+dy^2)*c) = exp(-dx^2*c) * exp(-dy^2*c)
and the sum over points becomes a matmul over the point axis:
    out[b].T = Gx^T @ (2*Bw*Gy)        (transposed-output formulation)

Sharding: batch (2) x point-shards (4): each of the 8 cores handles one batch
and 256 of the N=1024 points (two 128-point partition tiles, accumulated in
PSUM across the two tiles).  Each core emits an unclipped partial
[2(wc),128(wp),256(h)]; the host sums the 4 shards per batch, transposes, and
clips.

Device per core:
  - DMA in pp[128,4] (stimulation + sigma scale, one column per point-tile)
    and sqd0/sqd1[128,512] = -0.5*[((xs-vx)*d2p)^2 | ((ys-vy)*d2p)^2].
  - Neuron math on [128,2] tiles (sigmoid via 1/(1+exp(-x)) so only the
    exp_and_others ACT table set is ever loaded; no sqrt needed because
    max(sqrt(v),1)^2 == max(v,1) for v>=0).
  - Per point-tile: one fused Exp [128,512] -> Gx|Gy in fp32r (rounded fp32:
    full-rate matmuls when the moving dim is >=256, near-fp32 accuracy,
    fp32 exponent range), scale Gy by 2*Bw, two PSUM-accumulating matmuls
    (w-chunks), copy out via DVE/ACT in parallel, DMA on both HWDGE rings.
"""

import numpy as np

import concourse.bass as bass
import concourse.bacc as bacc
import concourse.mybir as mybir
from concourse import tile
from concourse.bass_utils import run_bass_kernel_spmd

N_CORES = 8
NSHARDS = 4        # point shards per batch
PPC = 256          # points per core
NPT = 128          # points per partition tile
B = 2
H = W = 256

SPREAD = 0.000675
R2S = 0.5
SLOPE = 19152642.5
HALF = 1.057e-07
RHEO = 2.39e-05
FREQ = 300.0
PW = 0.00017
I_SCALE = 8e-05

F32 = mybir.dt.float32
F16 = mybir.dt.float16
F32R = mybir.dt.float32r
ALU = mybir.AluOpType
ACT = mybir.ActivationFunctionType

_NC = None


def _build_nc():
    nc = bacc.Bacc(None, target_bir_lowering=False, debug=False,
                   num_devices=N_CORES)
    pp = nc.dram_tensor("pp", [NPT, 4], F32, kind="ExternalInput")
    sqd0 = nc.dram_tensor("sqd0", [NPT, 2 * W], F32, kind="ExternalInput")
    sqd1 = nc.dram_tensor("sqd1", [NPT, 2 * W], F32, kind="ExternalInput")
    partial = nc.dram_tensor("partial", [2, 128, W], F32, kind="ExternalOutput")

    with tile.TileContext(nc) as tc:
        with (
            tc.tile_pool(name="const", bufs=1) as cpool,
            tc.tile_pool(name="work", bufs=2) as wpool,
            tc.tile_pool(name="obuf", bufs=2) as opool,
            tc.tile_pool(name="psum", bufs=2, space="PSUM") as psum,
        ):
            ppt = cpool.tile([NPT, 4], F32)
            nc.sync.dma_start(ppt[:], pp[:])
            sqdt = [cpool.tile([NPT, 2 * W], F32, tag=f"sqd{p}", name=f"sqdt{p}") for p in range(2)]
            nc.sync.dma_start(sqdt[0][:], sqd0[:])
            nc.sync.dma_start(sqdt[1][:], sqd1[:])

            # Cold-start absorber: a throwaway matmul on data that is ready
            # long before the real ones (PE is idle until ~3.7us otherwise),
            # so the real matmuls run at the warm clock with no LDW stall.
            wdum = cpool.tile([NPT, 2], F32)
            nc.vector.memset(wdum[:], 0.0)
            psd = psum.tile([2, 64], F32, tag="psd", name="psd", bufs=1)
            nc.tensor.matmul(psd[:], wdum[:], sqdt[0][:, 0:64], start=True, stop=True)

            # -- Bw = sigmoid(SLOPE*(Q-HALF)) via 1/(1+exp(-t)), one column
            # per point-tile.  t = SLOPE*PW*FREQ*I_SCALE*ieff - SLOPE*HALF
            ieff = cpool.tile([NPT, 2], F32)
            nc.vector.tensor_scalar(
                ieff[:], ppt[:, 0:2], RHEO / I_SCALE, 0.0, ALU.subtract, ALU.max
            )
            targ = cpool.tile([NPT, 2], F32)
            nc.vector.tensor_scalar(
                targ[:], ieff[:],
                float(-SLOPE * PW * FREQ * I_SCALE), float(SLOPE * HALF),
                ALU.mult, ALU.add,
            )
            e = cpool.tile([NPT, 2], F32)
            nc.scalar.activation(e[:], targ[:], ACT.Exp)
            ope = cpool.tile([NPT, 2], F32)
            nc.vector.tensor_scalar(ope[:], e[:], 1.0, None, ALU.add)
            bw = cpool.tile([NPT, 2], F32)
            nc.vector.reciprocal(bw[:], ope[:])

            # -- negc = 1/max(stim*minv2sc, 1); the -0.5 factor is baked into
            # sqd on the host, so exp(sqd * negc) is the Gaussian directly.
            v0 = cpool.tile([NPT, 2], F32)
            nc.vector.tensor_tensor(v0[:], ppt[:, 0:2], ppt[:, 2:4], ALU.mult)
            v = cpool.tile([NPT, 2], F32)
            nc.vector.tensor_scalar(v[:], v0[:], 1.0, None, ALU.max)
            negc = cpool.tile([NPT, 2], F32)
            nc.vector.reciprocal(negc[:], v[:])

            # Per point-tile Gaussians; PSUM accumulates over the two tiles.
            pss = [psum.tile([128, W], F32, tag=f"ps{wc}", name=f"ps{wc}") for wc in range(2)]
            for p in range(2):
                gxy = wpool.tile([NPT, 2 * W], F32R, tag="gxy")
                nc.scalar.activation(
                    gxy[:], sqdt[p][:], ACT.Exp, scale=negc[:, p:p + 1],
                )
                gys = wpool.tile([NPT, W], F32R, tag="gys")
                nc.vector.tensor_scalar(
                    gys[:], gxy[:, W:2 * W], bw[:, p:p + 1], 2.0, ALU.mult, ALU.mult
                )
                # Transposed formulation: stationary = Gx chunk (ready before
                # gys), moving = gys; LDWEIGHTS stays off the critical path.
                for wc in range(2):
                    nc.tensor.matmul(
                        pss[wc][:],
                        gxy[:, wc * 128:(wc + 1) * 128],
                        gys[:],
                        start=(p == 0), stop=(p == 1),
                    )
            for wc in range(2):
                ob = opool.tile([128, W], F32)
                # Copies split across DVE and ACT so they run concurrently;
                # each DMA goes out on its issuer's HWDGE ring.
                if wc == 0:
                    nc.vector.tensor_copy(ob[:], pss[wc][:])
                    nc.sync.dma_start(partial[wc], ob[:])
                else:
                    nc.scalar.copy(ob[:], pss[wc][:])
                    nc.scalar.dma_start(partial[wc], ob[:])
    nc.compile()
    return nc


def _get_nc():
    global _NC
    if _NC is None:
        _NC = _build_nc()
    return _NC


def make_in_maps(stimulation, vx, vy, M, px, py, idx):
    stimulation = np.asarray(stimulation, dtype=np.float32)
    vx = np.asarray(vx, dtype=np.float32)
    vy = np.asarray(vy, dtype=np.float32)
    M = np.asarray(M, dtype=np.float32)
    px = np.asarray(px, dtype=np.float32)
    py = np.asarray(py, dtype=np.float32)
    idx = np.asarray(idx)

    fov = np.float32(px.max())
    deg2pix = np.float32(W) / (fov * np.float32(2.0))
    xs = px[0, :]            # px[h,w] = xs[w]
    ys = py[:, 0]            # py[h,w] = ys[h]
    flat = stimulation.reshape(B, -1)[:, idx]          # [B, N]
    minv2sc = (I_SCALE / SPREAD) * (R2S * deg2pix / M) ** 2  # [N]

    def sqd_for(sl):
        dx = (xs[None, :] - vx[sl, None]) * deg2pix    # [NPT, W]
        dy = (ys[None, :] - vy[sl, None]) * deg2pix    # [NPT, H]
        # -0.5 baked in: exponent = sqd * (1/max(sigma_px^2, 1))
        out = np.concatenate([dx * dx, dy * dy], axis=1) * np.float32(-0.5)
        return np.ascontiguousarray(out, dtype=np.float32)

    in_maps = []
    for c in range(N_CORES):
        b, s = divmod(c, NSHARDS)
        sl0 = slice(s * PPC, s * PPC + NPT)
        sl1 = slice(s * PPC + NPT, (s + 1) * PPC)
        pp = np.zeros((NPT, 4), np.float32)
        pp[:, 0] = flat[b, sl0]
        pp[:, 1] = flat[b, sl1]
        pp[:, 2] = minv2sc[sl0]
        pp[:, 3] = minv2sc[sl1]
        in_maps.append({
            "pp": pp,
            "sqd0": sqd_for(sl0),
            "sqd1": sqd_for(sl1),
        })
    return in_maps


def combine(results):
    acc = np.zeros((B, H, W), np.float32)
    for c, r in enumerate(results):
        b = c // NSHARDS
        # device emits out'[wc, wp, h]; out[b, h, wc*128+wp] = out'[...]
        p = r["partial"]
        acc[b] += p.transpose(2, 0, 1).reshape(H, W)
    return np.clip(acc, 0.0, 1.0)[:, None, :, :].astype(np.float32)


def kernel(stimulation, vx, vy, M, px, py, idx):
    nc = _get_nc()
    in_maps = make_in_maps(stimulation, vx, vy, M, px, py, idx)
    res = run_bass_kernel_spmd(nc, in_maps, list(range(N_CORES)))
    return combine(res.results)
